# revision 37
# baseline (speedup 1.0000x reference)
"""Trainium2 Bass kernel for nn_Attention2 (8-head encoder/decoder attention mix).

Reference computation (per full batch B=4096):
    enc_h  = relu(encoder_input @ W_enc + b_enc)               [B, 1024]
    heads  = relu(einsum('bh,khd->kbd', enc_h, W_heads) + b_heads)  [8, B, 1024]
    dec_H  = relu(decoder_input @ W_dec + b_dec)               [B, 1024]
    scores = sum(heads * dec_H, axis=2)                        [8, B]
    attn   = softmax(scores.T, axis=1)                         [B, 8]
    out    = einsum('kbd,bk->bd', heads, attn)                 [B, 1024]

Sharding: pure data-parallel over the batch dim across 8 NeuronCores
(B_loc = 512 per core, all params replicated, zero collectives).

Design notes (PE roofline: 608 matmuls of [128k x 128m x 512n] ~= 133us):
  - No bias matmuls: stage B/C PSUM tiles are PRE-FILLED with a broadcast
    bias row (ScalarE Copy for B, DVE copy for C) and the K-strip matmuls
    accumulate on top (start=False), pipelined 2 groups ahead of the
    drains so the PE never waits on ScalarE.
  - h-OUTER loop: W_heads (16 MB) streams during compute; head h is
    needed ~14.6us apart, far behind the ~355 GB/s aggregate DMA rate.
    All 8 heads resident in SBUF (128 KB/partition), fine-grained
    per-head tiles so head 0's matmuls only wait for head 0's DMA.
  - All bulk DMA rides the otherwise-idle sync queue in strict deadline
    order (queues share the 16 DMA engines with opaque arbitration, and
    a hw queue keeps only ~2 transfers in flight, so one ordered stream
    is both deterministic and near-optimal).
  - 8 junk warm-up matmuls during the initial DMA wait: they ramp the PE
    clock out of its low p-state AND touch every PSUM bank with a
    start=True matmul — banks used only by prefill+accumulate groups
    otherwise keep cold pending-zero state that silently drops the bias
    prefill on a cold core (caused an intermittent b-tile-0 error).
  - bf16 storage everywhere; measured-fast DVE ops (tensor_scalar 4x,
    tensor_tensor 2x; scalar_tensor_tensor 1x but fuses the score
    product+accumulate; tensor_tensor_reduce crashes the device - avoid).
  - Softmax finalize per b-tile interleaved into head 7 (divide split
    across ScalarE activation-scale and DVE) so only the last tile's
    ~4us chain trails the final matmul. Device output bf16 (host
    converts to f32), halving the output DMA.

Measured: 160.5us HW exec (baseline 176us), rel err 5.6e-3 (gate 2e-2).
"""

import numpy as np
from contextlib import ExitStack

N_CORES = 8
ENC_DIM, DEC_DIM, HID, HEADS, BATCH = 1024, 512, 1024, 8, 4096
B_LOC = BATCH // N_CORES          # 512 batch rows per core
P = 128                           # SBUF partitions
NCHUNK = 512                      # matmul moving free-dim (1 PSUM bank f32)
SCORE_SHIFT = 24.0                # scores measured in [14.2, 34.0]

KT_E = ENC_DIM // P               # 8 contraction tiles (enc dim)
KT_H = HID // P                   # 8 contraction tiles (hid dim)
KT_D = DEC_DIM // P               # 4 contraction tiles (dec dim)
MT = HID // P                     # 8 hid tiles (feature-major partitions)
BT = B_LOC // P                   # 4 batch tiles
NC_H = HID // NCHUNK              # 2 moving chunks over hid
QUARTER = 256                     # last-tile chain granularity

_cache = {}


def _build():
    import concourse.tile as tile
    from concourse import bacc, mybir

    f32 = mybir.dt.float32
    bf16 = mybir.dt.bfloat16
    Relu = mybir.ActivationFunctionType.Relu
    Exp = mybir.ActivationFunctionType.Exp
    Copy = mybir.ActivationFunctionType.Copy
    X = mybir.AxisListType.X
    mult = mybir.AluOpType.mult
    add = mybir.AluOpType.add
    vmax = mybir.AluOpType.max

    nc = bacc.Bacc("TRN2", target_bir_lowering=False, debug=False,
                   num_devices=N_CORES)

    xe_p = nc.dram_tensor("xe_p", [P, KT_E, B_LOC], bf16, kind="ExternalInput").ap()
    we_p = nc.dram_tensor("we_p", [P, KT_E, HID], bf16, kind="ExternalInput").ap()
    xd_p = nc.dram_tensor("xd_p", [P, KT_D, B_LOC], bf16, kind="ExternalInput").ap()
    wd_p = nc.dram_tensor("wd_p", [P, KT_D, HID], bf16, kind="ExternalInput").ap()
    wh_p = nc.dram_tensor("wh_p", [HEADS, P, KT_H, HID], bf16, kind="ExternalInput").ap()
    benc_pp = nc.dram_tensor("benc_pp", [P, MT], f32, kind="ExternalInput").ap()
    bh_bc = nc.dram_tensor("bh_bc", [HEADS, P, HID], bf16, kind="ExternalInput").ap()
    bd_bc = nc.dram_tensor("bd_bc", [P, HID], bf16, kind="ExternalInput").ap()
    out_d = nc.dram_tensor("out", [B_LOC, HID], bf16, kind="ExternalOutput").ap()

    with tile.TileContext(nc) as tc, ExitStack() as ctx:
        persist = ctx.enter_context(tc.tile_pool(name="persist", bufs=1))
        psums = ctx.enter_context(tc.tile_pool(name="psums", bufs=8, space="PSUM"))

        # --- persistent SBUF tiles ---
        WH = [persist.tile([P, KT_H, HID], bf16, tag=f"WH{h}", name=f"WH{h}")
              for h in range(HEADS)]
        BENC = persist.tile([P, MT], f32, tag="BENC", name="BENC")
        BD = persist.tile([P, HID], bf16, tag="BD", name="BD")
        BH = [persist.tile([P, HID], bf16, tag=f"BH{h}", name=f"BH{h}")
              for h in range(HEADS)]
        negC = persist.tile([P, 1], f32, tag="negC", name="negC")
        junk = persist.tile([P, NCHUNK], bf16, tag="junk", name="junk")
        nc.vector.memset(negC[:], -SCORE_SHIFT)
        nc.vector.memset(junk[:], 0.5)
        ench = [persist.tile([P, B_LOC], bf16, tag=f"ench{m}", name=f"ench{m}")
                for m in range(MT)]
        dec_bm = [persist.tile([P, HID], bf16, tag=f"dec{b}", name=f"dec{b}")
                  for b in range(BT)]
        e_all = [persist.tile([P, HEADS], f32, tag=f"eall{b}", name=f"eall{b}")
                 for b in range(BT)]
        oacc = [persist.tile([P, HID], bf16, tag=f"oacc{b}", name=f"oacc{b}")
                for b in range(BT)]

        poolA = tc.tile_pool(name="poolA", bufs=1)
        pa = poolA.__enter__()
        XEh = [pa.tile([P, KT_E // 2, B_LOC], bf16, tag=f"XEh{i}", name=f"XEh{i}")
               for i in range(2)]
        WEh = [pa.tile([P, KT_E // 2, HID], bf16, tag=f"WEh{i}", name=f"WEh{i}")
               for i in range(2)]
        XD = pa.tile([P, KT_D, B_LOC], bf16, tag="XD", name="XD")
        WD = pa.tile([P, KT_D, HID], bf16, tag="WD", name="WD")

        def XEk(k):
            return XEh[k // (KT_E // 2)][:, k % (KT_E // 2), :]

        def WEk(k):
            return WEh[k // (KT_E // 2)][:, k % (KT_E // 2), :]

        # --- t=0 DMA issue: one queue (sync) in strict global deadline
        # order — queues race for the same ~355 GB/s of shared DMA
        # engines with opaque arbitration, so one ordered stream is both
        # deterministic and optimal. Only the tiny BENC rides scalar.
        nc.scalar.dma_start(BENC[:], benc_pp)
        nc.sync.dma_start(XEh[0][:], xe_p[:, 0:KT_E // 2, :])
        nc.sync.dma_start(WEh[0][:], we_p[:, 0:KT_E // 2, :])
        nc.sync.dma_start(XEh[1][:], xe_p[:, KT_E // 2:KT_E, :])
        nc.sync.dma_start(WEh[1][:], we_p[:, KT_E // 2:KT_E, :])
        nc.sync.dma_start(XD[:], xd_p)
        nc.sync.dma_start(WD[:], wd_p)
        nc.sync.dma_start(BD[:], bd_bc)
        nc.sync.dma_start(BH[0][:], bh_bc[0])
        nc.sync.dma_start(BH[1][:], bh_bc[1])
        nc.sync.dma_start(WH[0][:], wh_p[0])
        nc.sync.dma_start(BH[2][:], bh_bc[2])
        nc.sync.dma_start(BH[3][:], bh_bc[3])
        nc.sync.dma_start(WH[1][:], wh_p[1])
        nc.sync.dma_start(WH[2][:], wh_p[2])
        for h in range(4, HEADS):
            nc.sync.dma_start(BH[h][:], bh_bc[h])
        for h in range(3, HEADS):
            nc.sync.dma_start(WH[h][:], wh_p[h])

        # --- PE p-state warm-up on junk data during the DMA wait. One
        # start=True matmul per PSUM bank: banks later used ONLY by
        # prefill+accumulate groups otherwise keep cold pending-zero
        # state that silently drops the bias prefill on first use.
        for i in range(15):
            wps = psums.tile([P, NCHUNK], f32, tag="mm", name="ps")
            nc.tensor.matmul(wps[:], junk[:, :P], junk[:],
                             start=True, stop=True)

        # ---- group list: stage C (8 groups), then stage B h-outer ----
        groups = []
        for b in range(BT):
            for n in range(NC_H):
                groups.append(("C", None, b, n))
        for h in range(HEADS):
            for b in range(BT):
                for n in range(NC_H):
                    groups.append(("B", h, b, n))
        NG = len(groups)
        ps_of = {}

        def prefill(g):
            kind, h, b, n = groups[g]
            ps = psums.tile([P, NCHUNK], f32, tag="mm", name="ps")
            ncol = slice(n * NCHUNK, (n + 1) * NCHUNK)
            if kind == "C":
                nc.vector.tensor_copy(ps[:], BD[:, ncol])
            else:
                nc.scalar.activation(ps[:], BH[h][:, ncol], Copy)
            ps_of[g] = ps

        def emit_matmuls(g):
            kind, h, b, n = groups[g]
            ps = ps_of[g]
            ncol = slice(n * NCHUNK, (n + 1) * NCHUNK)
            bcol = slice(b * P, (b + 1) * P)
            if kind == "C":
                for k in range(KT_D):
                    nc.tensor.matmul(ps[:], XD[:, k, bcol], WD[:, k, ncol],
                                     start=False, stop=(k == KT_D - 1),
                                     skip_group_check=True)
            else:
                for k in range(KT_H):
                    nc.tensor.matmul(ps[:], ench[k][:, bcol],
                                     WH[h][:, k, ncol],
                                     start=False, stop=(k == KT_H - 1),
                                     skip_group_check=True)

        # ---- Stage A: enc trunk, feature-major, 2 waves of 4 m-tiles ----
        for wave in range(2):
            mset = range(wave * MT // 2, (wave + 1) * MT // 2)
            pss = {}
            for m in mset:
                pss[m] = psums.tile([P, B_LOC], f32, tag="mm", name="ps")
            for k in range(KT_E):
                for m in mset:
                    nc.tensor.matmul(pss[m][:], WEk(k)[:, m * P:(m + 1) * P],
                                     XEk(k),
                                     start=(k == 0), stop=(k == KT_E - 1))
            for m in mset:
                nc.scalar.activation(ench[m][:], pss[m][:], Relu,
                                     bias=BENC[:, m:m + 1], scale=1.0)
            if wave == 0:
                prefill(0)
                prefill(1)

        # ---- Stage C: DVE handles bias prefill + relu drain ----
        for g in range(BT * NC_H):
            kind, h, b, n = groups[g]
            ncol = slice(n * NCHUNK, (n + 1) * NCHUNK)
            emit_matmuls(g)
            if g + 2 < NG:
                prefill(g + 2)
            ps_cur = ps_of.pop(g)
            nc.vector.tensor_scalar(dec_bm[b][:, ncol], ps_cur[:], 0.0, None,
                                    op0=vmax)

        poolA.__exit__(None, None, None)

        # ---- Stage B: h-outer; finalize interleaved into head 7 ----
        head_pool = ctx.enter_context(tc.tile_pool(name="head", bufs=3))
        scratch = ctx.enter_context(tc.tile_pool(name="scratch", bufs=4))
        fin = ctx.enter_context(tc.tile_pool(name="fin", bufs=2))

        head_t = None
        for g in range(BT * NC_H, NG):
            kind, h, b, n = groups[g]
            ncol = slice(n * NCHUNK, (n + 1) * NCHUNK)
            last = h == HEADS - 1 and b == BT - 1
            emit_matmuls(g)
            if g + 2 < NG:
                prefill(g + 2)
            ps_cur = ps_of.pop(g)
            if n == 0:
                head_t = head_pool.tile([P, HID], bf16, tag="head", name="head")
            if not (last and n == NC_H - 1):
                nc.scalar.activation(head_t[:, ncol], ps_cur[:], Relu)
            else:
                # very last chunk: relu in quarters so the score chain
                # starts earlier
                for q in range(2):
                    qcol = slice(NCHUNK + q * QUARTER, NCHUNK + (q + 1) * QUARTER)
                    pcol = slice(q * QUARTER, (q + 1) * QUARTER)
                    nc.scalar.activation(head_t[:, qcol], ps_cur[:, pcol], Relu)
            if n != NC_H - 1:
                continue

            # ---- score + exp + out-accumulate for (h, b) ----
            prod = scratch.tile([P, HID], bf16, tag="prod", name="prod")
            s_col = scratch.tile([P, 1], f32, tag="scol", name="scol")
            e_sc = e_all[b][:, h:h + 1]
            if not last:
                nc.vector.scalar_tensor_tensor(
                    prod[:], head_t[:], 1.0, dec_bm[b][:],
                    op0=mult, op1=mult, accum_out=s_col[:])
                nc.scalar.activation(e_sc, s_col[:], Exp,
                                     bias=negC[:], scale=1.0)
                if h == 0:
                    nc.vector.tensor_scalar(oacc[b][:], head_t[:], e_sc, None,
                                            op0=mult)
                else:
                    nc.vector.scalar_tensor_tensor(
                        oacc[b][:], head_t[:], e_sc, oacc[b][:],
                        op0=mult, op1=add)
            else:
                # last (h,b): quartered score, then fused finalize
                sq = [scratch.tile([P, 1], f32, tag=f"sq{q}", name=f"sq{q}")
                      for q in range(4)]
                for q in range(4):
                    qcol = slice(q * QUARTER, (q + 1) * QUARTER)
                    nc.vector.scalar_tensor_tensor(
                        prod[:, qcol], head_t[:, qcol], 1.0,
                        dec_bm[b][:, qcol],
                        op0=mult, op1=mult, accum_out=sq[q][:])
                nc.vector.tensor_add(sq[0][:], sq[0][:], sq[1][:])
                nc.vector.tensor_add(sq[2][:], sq[2][:], sq[3][:])
                nc.vector.tensor_add(s_col[:], sq[0][:], sq[2][:])
                nc.scalar.activation(e_sc, s_col[:], Exp,
                                     bias=negC[:], scale=1.0)

            # ---- finalize batch tile b right after its head-7 part ----
            if h == HEADS - 1:
                s_sum = scratch.tile([P, 1], f32, tag="ssum", name="ssum")
                rinv = scratch.tile([P, 1], f32, tag="rinv", name="rinv")
                nc.vector.reduce_sum(s_sum[:], e_all[b][:], axis=X)
                nc.vector.reciprocal(rinv[:], s_sum[:])
                out_f = fin.tile([P, HID], bf16, tag="outf", name="outf")
                brow = slice(b * P, (b + 1) * P)
                if not last:
                    # split the divide across ScalarE (has slack) and DVE
                    h0col = slice(0, NCHUNK)
                    h1col = slice(NCHUNK, HID)
                    nc.scalar.activation(out_f[:, h0col], oacc[b][:, h0col],
                                         Copy, scale=rinv[:])
                    nc.sync.dma_start(out_d[brow, h0col], out_f[:, h0col])
                    nc.vector.tensor_scalar(out_f[:, h1col],
                                            oacc[b][:, h1col],
                                            rinv[:], None, op0=mult)
                    nc.sync.dma_start(out_d[brow, h1col], out_f[:, h1col])
                else:
                    # fused: out = oacc*rinv + head7*(e7*rinv); divide on
                    # ScalarE, fused multiply-add on DVE, quartered so the
                    # two engines and the out-DMA pipeline
                    q7 = scratch.tile([P, 1], f32, tag="q7", name="q7")
                    nc.vector.tensor_scalar(q7[:], e_sc, rinv[:], None,
                                            op0=mult)
                    for q in range(4):
                        qcol = slice(q * QUARTER, (q + 1) * QUARTER)
                        nc.scalar.activation(out_f[:, qcol], oacc[b][:, qcol],
                                             Copy, scale=rinv[:])
                        nc.vector.scalar_tensor_tensor(
                            out_f[:, qcol], head_t[:, qcol], q7[:],
                            out_f[:, qcol], op0=mult, op1=add)
                        nc.sync.dma_start(out_d[brow, qcol], out_f[:, qcol])

    nc.compile()
    return nc


def _get_nc():
    if "nc" not in _cache:
        _cache["nc"] = _build()
    return _cache["nc"]


def build_in_maps(encoder_input, decoder_input, W_enc, b_enc, W_heads,
                  b_heads, W_dec, b_dec):
    import ml_dtypes
    bf = ml_dtypes.bfloat16

    def cast(a):
        return np.ascontiguousarray(np.asarray(a, dtype=np.float32)).astype(bf)

    xe = np.asarray(encoder_input, np.float32)     # [4096, 1024]
    xd = np.asarray(decoder_input, np.float32)     # [4096, 512]
    W_enc = np.asarray(W_enc, np.float32)
    W_dec = np.asarray(W_dec, np.float32)
    W_heads = np.asarray(W_heads, np.float32)

    we_p = cast(np.ascontiguousarray(
        W_enc.reshape(KT_E, P, HID).transpose(1, 0, 2)))           # [128,8,1024]
    wd_p = cast(np.ascontiguousarray(
        W_dec.reshape(KT_D, P, HID).transpose(1, 0, 2)))           # [128,4,1024]
    wh_p = cast(np.ascontiguousarray(
        W_heads.reshape(HEADS, KT_H, P, HID).transpose(0, 2, 1, 3)))  # [8,128,8,1024]

    benc_pp = np.ascontiguousarray(
        np.asarray(b_enc, np.float32).reshape(MT, P).T)            # [128, 8]
    bh_bc = cast(np.broadcast_to(
        np.asarray(b_heads, np.float32)[:, None, :], (HEADS, P, HID)))
    bd_bc = cast(np.broadcast_to(
        np.asarray(b_dec, np.float32)[None, :], (P, HID)))

    shared = {
        "we_p": we_p,
        "wd_p": wd_p,
        "wh_p": wh_p,
        "benc_pp": benc_pp,
        "bh_bc": bh_bc,
        "bd_bc": bd_bc,
    }
    in_maps = []
    for c in range(N_CORES):
        sl = slice(c * B_LOC, (c + 1) * B_LOC)
        m = dict(shared)
        m["xe_p"] = cast(np.ascontiguousarray(
            xe[sl].T.reshape(KT_E, P, B_LOC).transpose(1, 0, 2)))
        m["xd_p"] = cast(np.ascontiguousarray(
            xd[sl].T.reshape(KT_D, P, B_LOC).transpose(1, 0, 2)))
        in_maps.append(m)
    return in_maps


def kernel(encoder_input, decoder_input, W_enc, b_enc, W_heads, b_heads,
           W_dec, b_dec):
    from concourse.bass_utils import run_bass_kernel_spmd

    nc = _get_nc()
    in_maps = build_in_maps(encoder_input, decoder_input, W_enc, b_enc,
                            W_heads, b_heads, W_dec, b_dec)
    res = run_bass_kernel_spmd(nc, in_maps, list(range(N_CORES)))
    out = np.concatenate(
        [np.asarray(res.results[c]["out"]).astype(np.float32)
         for c in range(N_CORES)], axis=0)
    return out


# revision 38
# speedup vs baseline: 1.1875x; 1.1875x over previous
"""Trainium2 Bass kernel for nn_Attention2 (8-head encoder/decoder attention mix).

Reference computation (per full batch B=4096):
    enc_h  = relu(encoder_input @ W_enc + b_enc)               [B, 1024]
    heads  = relu(einsum('bh,khd->kbd', enc_h, W_heads) + b_heads)  [8, B, 1024]
    dec_H  = relu(decoder_input @ W_dec + b_dec)               [B, 1024]
    scores = sum(heads * dec_H, axis=2)                        [8, B]
    attn   = softmax(scores.T, axis=1)                         [B, 8]
    out    = einsum('kbd,bk->bd', heads, attn)                 [B, 1024]

Sharding: pure data-parallel over the batch dim across 8 NeuronCores
(B_loc = 512 per core, all params replicated, zero collectives).

Design notes (PE roofline: 608 matmuls of [128k x 128m x 512n] ~= 133us):
  - No bias matmuls: stage B/C PSUM tiles are PRE-FILLED with a broadcast
    bias row (ScalarE Copy for B, DVE copy for C) and the K-strip matmuls
    accumulate on top (start=False), pipelined 2 groups ahead of the
    drains so the PE never waits on ScalarE.
  - h-OUTER loop: W_heads (16 MB) streams during compute; head h is
    needed ~14.6us apart, far behind the ~355 GB/s aggregate DMA rate.
    All 8 heads resident in SBUF (128 KB/partition), fine-grained
    per-head tiles so head 0's matmuls only wait for head 0's DMA.
  - All bulk DMA rides the otherwise-idle sync queue in strict deadline
    order (queues share the 16 DMA engines with opaque arbitration, and
    a hw queue keeps only ~2 transfers in flight, so one ordered stream
    is both deterministic and near-optimal).
  - 8 junk warm-up matmuls during the initial DMA wait: they ramp the PE
    clock out of its low p-state AND touch every PSUM bank with a
    start=True matmul — banks used only by prefill+accumulate groups
    otherwise keep cold pending-zero state that silently drops the bias
    prefill on a cold core (caused an intermittent b-tile-0 error).
  - bf16 storage everywhere; measured-fast DVE ops (tensor_scalar 4x,
    tensor_tensor 2x; scalar_tensor_tensor 1x but fuses the score
    product+accumulate; tensor_tensor_reduce crashes the device - avoid).
  - Softmax finalize per b-tile interleaved into head 7 (divide split
    across ScalarE activation-scale and DVE) so only the last tile's
    ~4us chain trails the final matmul. Device output bf16 (host
    converts to f32), halving the output DMA.

Measured: 160.5us HW exec (baseline 176us), rel err 5.6e-3 (gate 2e-2).
"""

import numpy as np
from contextlib import ExitStack

N_CORES = 8
ENC_DIM, DEC_DIM, HID, HEADS, BATCH = 1024, 512, 1024, 8, 4096
B_LOC = BATCH // N_CORES          # 512 batch rows per core
P = 128                           # SBUF partitions
NCHUNK = 512                      # matmul moving free-dim (1 PSUM bank f32)
SCORE_SHIFT = 24.0                # scores measured in [14.2, 34.0]

KT_E = ENC_DIM // P               # 8 contraction tiles (enc dim)
KT_H = HID // P                   # 8 contraction tiles (hid dim)
KT_D = DEC_DIM // P               # 4 contraction tiles (dec dim)
MT = HID // P                     # 8 hid tiles (feature-major partitions)
BT = B_LOC // P                   # 4 batch tiles
NC_H = HID // NCHUNK              # 2 moving chunks over hid
QUARTER = 256                     # last-tile chain granularity

_cache = {}


def _build():
    import concourse.tile as tile
    from concourse import bacc, mybir

    f32 = mybir.dt.float32
    bf16 = mybir.dt.bfloat16
    Relu = mybir.ActivationFunctionType.Relu
    Exp = mybir.ActivationFunctionType.Exp
    Copy = mybir.ActivationFunctionType.Copy
    X = mybir.AxisListType.X
    mult = mybir.AluOpType.mult
    add = mybir.AluOpType.add
    vmax = mybir.AluOpType.max

    nc = bacc.Bacc("TRN2", target_bir_lowering=False, debug=False,
                   num_devices=N_CORES)

    xe_p = nc.dram_tensor("xe_p", [P, KT_E, B_LOC], bf16, kind="ExternalInput").ap()
    we_p = nc.dram_tensor("we_p", [P, KT_E, HID], bf16, kind="ExternalInput").ap()
    xd_p = nc.dram_tensor("xd_p", [P, KT_D, B_LOC], bf16, kind="ExternalInput").ap()
    wd_p = nc.dram_tensor("wd_p", [P, KT_D, HID], bf16, kind="ExternalInput").ap()
    wh_p = nc.dram_tensor("wh_p", [HEADS, P, KT_H, HID], bf16, kind="ExternalInput").ap()
    benc_pp = nc.dram_tensor("benc_pp", [P, MT], f32, kind="ExternalInput").ap()
    bh_bc = nc.dram_tensor("bh_bc", [HEADS, P, HID], bf16, kind="ExternalInput").ap()
    bd_bc = nc.dram_tensor("bd_bc", [P, HID], bf16, kind="ExternalInput").ap()
    out_d = nc.dram_tensor("out", [B_LOC, HID], bf16, kind="ExternalOutput").ap()

    with tile.TileContext(nc) as tc, ExitStack() as ctx:
        persist = ctx.enter_context(tc.tile_pool(name="persist", bufs=1))
        psums = ctx.enter_context(tc.tile_pool(name="psums", bufs=8, space="PSUM"))

        # --- persistent SBUF tiles ---
        WH = [persist.tile([P, KT_H, HID], bf16, tag=f"WH{h}", name=f"WH{h}")
              for h in range(HEADS)]
        BENC = persist.tile([P, MT], f32, tag="BENC", name="BENC")
        BD = persist.tile([P, HID], bf16, tag="BD", name="BD")
        BH = [persist.tile([P, HID], bf16, tag=f"BH{h}", name=f"BH{h}")
              for h in range(HEADS)]
        negC = persist.tile([P, 1], f32, tag="negC", name="negC")
        junk = persist.tile([P, NCHUNK], bf16, tag="junk", name="junk")
        nc.vector.memset(negC[:], -SCORE_SHIFT)
        nc.vector.memset(junk[:], 0.5)
        ench = [persist.tile([P, B_LOC], bf16, tag=f"ench{m}", name=f"ench{m}")
                for m in range(MT)]
        dec_bm = [persist.tile([P, HID], bf16, tag=f"dec{b}", name=f"dec{b}")
                  for b in range(BT)]
        e_all = [persist.tile([P, HEADS], f32, tag=f"eall{b}", name=f"eall{b}")
                 for b in range(BT)]
        oacc = [persist.tile([P, HID], bf16, tag=f"oacc{b}", name=f"oacc{b}")
                for b in range(BT)]

        poolA = tc.tile_pool(name="poolA", bufs=1)
        pa = poolA.__enter__()
        XEh = [pa.tile([P, KT_E // 2, B_LOC], bf16, tag=f"XEh{i}", name=f"XEh{i}")
               for i in range(2)]
        WEh = [pa.tile([P, KT_E // 2, HID], bf16, tag=f"WEh{i}", name=f"WEh{i}")
               for i in range(2)]
        XD = pa.tile([P, KT_D, B_LOC], bf16, tag="XD", name="XD")
        WD = pa.tile([P, KT_D, HID], bf16, tag="WD", name="WD")

        def XEk(k):
            return XEh[k // (KT_E // 2)][:, k % (KT_E // 2), :]

        def WEk(k):
            return WEh[k // (KT_E // 2)][:, k % (KT_E // 2), :]

        # --- t=0 DMA issue: one queue (sync) in strict global deadline
        # order — queues race for the same ~355 GB/s of shared DMA
        # engines with opaque arbitration, so one ordered stream is both
        # deterministic and optimal. Only the tiny BENC rides scalar.
        nc.scalar.dma_start(BENC[:], benc_pp)
        nc.sync.dma_start(XEh[0][:], xe_p[:, 0:KT_E // 2, :])
        nc.sync.dma_start(WEh[0][:], we_p[:, 0:KT_E // 2, :])
        nc.sync.dma_start(XEh[1][:], xe_p[:, KT_E // 2:KT_E, :])
        nc.sync.dma_start(WEh[1][:], we_p[:, KT_E // 2:KT_E, :])
        nc.sync.dma_start(XD[:], xd_p)
        nc.sync.dma_start(WD[:], wd_p)
        nc.sync.dma_start(BD[:], bd_bc)
        nc.sync.dma_start(BH[0][:], bh_bc[0])
        nc.sync.dma_start(BH[1][:], bh_bc[1])
        nc.sync.dma_start(WH[0][:], wh_p[0])
        nc.sync.dma_start(BH[2][:], bh_bc[2])
        nc.sync.dma_start(BH[3][:], bh_bc[3])
        nc.sync.dma_start(WH[1][:], wh_p[1])
        nc.sync.dma_start(WH[2][:], wh_p[2])
        for h in range(4, HEADS):
            nc.sync.dma_start(BH[h][:], bh_bc[h])
        for h in range(3, HEADS):
            nc.sync.dma_start(WH[h][:], wh_p[h])

        # --- PE p-state warm-up on junk data during the DMA wait. One
        # start=True matmul per PSUM bank: banks later used ONLY by
        # prefill+accumulate groups otherwise keep cold pending-zero
        # state that silently drops the bias prefill on first use.
        for i in range(8):
            wps = psums.tile([P, NCHUNK], f32, tag="mm", name="ps")
            nc.tensor.matmul(wps[:], junk[:, :P], junk[:],
                             start=True, stop=True)

        # ---- group list: stage C (8 groups), then stage B h-outer ----
        groups = []
        for b in range(BT):
            for n in range(NC_H):
                groups.append(("C", None, b, n))
        for h in range(HEADS):
            for b in range(BT):
                for n in range(NC_H):
                    groups.append(("B", h, b, n))
        NG = len(groups)
        ps_of = {}

        def prefill(g):
            kind, h, b, n = groups[g]
            ps = psums.tile([P, NCHUNK], f32, tag="mm", name="ps")
            ncol = slice(n * NCHUNK, (n + 1) * NCHUNK)
            if kind == "C":
                nc.vector.tensor_copy(ps[:], BD[:, ncol])
            else:
                nc.scalar.activation(ps[:], BH[h][:, ncol], Copy)
            ps_of[g] = ps

        def emit_matmuls(g):
            kind, h, b, n = groups[g]
            ps = ps_of[g]
            ncol = slice(n * NCHUNK, (n + 1) * NCHUNK)
            bcol = slice(b * P, (b + 1) * P)
            if kind == "C":
                for k in range(KT_D):
                    nc.tensor.matmul(ps[:], XD[:, k, bcol], WD[:, k, ncol],
                                     start=False, stop=(k == KT_D - 1),
                                     skip_group_check=True)
            else:
                for k in range(KT_H):
                    nc.tensor.matmul(ps[:], ench[k][:, bcol],
                                     WH[h][:, k, ncol],
                                     start=False, stop=(k == KT_H - 1),
                                     skip_group_check=True)

        # ---- Stage A: enc trunk, feature-major, 2 waves of 4 m-tiles ----
        for wave in range(2):
            mset = range(wave * MT // 2, (wave + 1) * MT // 2)
            pss = {}
            for m in mset:
                pss[m] = psums.tile([P, B_LOC], f32, tag="mm", name="ps")
            for k in range(KT_E):
                for m in mset:
                    nc.tensor.matmul(pss[m][:], WEk(k)[:, m * P:(m + 1) * P],
                                     XEk(k),
                                     start=(k == 0), stop=(k == KT_E - 1))
            for m in mset:
                nc.scalar.activation(ench[m][:], pss[m][:], Relu,
                                     bias=BENC[:, m:m + 1], scale=1.0)
            if wave == 0:
                prefill(0)
                prefill(1)

        # ---- Stage C: DVE handles bias prefill + relu drain ----
        for g in range(BT * NC_H):
            kind, h, b, n = groups[g]
            ncol = slice(n * NCHUNK, (n + 1) * NCHUNK)
            emit_matmuls(g)
            if g + 2 < NG:
                prefill(g + 2)
            ps_cur = ps_of.pop(g)
            nc.vector.tensor_scalar(dec_bm[b][:, ncol], ps_cur[:], 0.0, None,
                                    op0=vmax)

        poolA.__exit__(None, None, None)

        # ---- Stage B: h-outer; finalize interleaved into head 7 ----
        head_pool = ctx.enter_context(tc.tile_pool(name="head", bufs=3))
        scratch = ctx.enter_context(tc.tile_pool(name="scratch", bufs=4))
        fin = ctx.enter_context(tc.tile_pool(name="fin", bufs=2))

        head_t = None
        for g in range(BT * NC_H, NG):
            kind, h, b, n = groups[g]
            ncol = slice(n * NCHUNK, (n + 1) * NCHUNK)
            last = h == HEADS - 1 and b == BT - 1
            emit_matmuls(g)
            if g + 2 < NG:
                prefill(g + 2)
            ps_cur = ps_of.pop(g)
            if n == 0:
                head_t = head_pool.tile([P, HID], bf16, tag="head", name="head")
            if not (last and n == NC_H - 1):
                nc.scalar.activation(head_t[:, ncol], ps_cur[:], Relu)
            else:
                # very last chunk: relu in quarters so the score chain
                # starts earlier
                for q in range(2):
                    qcol = slice(NCHUNK + q * QUARTER, NCHUNK + (q + 1) * QUARTER)
                    pcol = slice(q * QUARTER, (q + 1) * QUARTER)
                    nc.scalar.activation(head_t[:, qcol], ps_cur[:, pcol], Relu)
            if n != NC_H - 1:
                continue

            # ---- score + exp + out-accumulate for (h, b) ----
            prod = scratch.tile([P, HID], bf16, tag="prod", name="prod")
            s_col = scratch.tile([P, 1], f32, tag="scol", name="scol")
            e_sc = e_all[b][:, h:h + 1]
            if not last:
                nc.vector.scalar_tensor_tensor(
                    prod[:], head_t[:], 1.0, dec_bm[b][:],
                    op0=mult, op1=mult, accum_out=s_col[:])
                nc.scalar.activation(e_sc, s_col[:], Exp,
                                     bias=negC[:], scale=1.0)
                if h == 0:
                    nc.vector.tensor_scalar(oacc[b][:], head_t[:], e_sc, None,
                                            op0=mult)
                else:
                    nc.vector.scalar_tensor_tensor(
                        oacc[b][:], head_t[:], e_sc, oacc[b][:],
                        op0=mult, op1=add)
            else:
                # last (h,b): quartered score, then fused finalize
                sq = [scratch.tile([P, 1], f32, tag=f"sq{q}", name=f"sq{q}")
                      for q in range(4)]
                for q in range(4):
                    qcol = slice(q * QUARTER, (q + 1) * QUARTER)
                    nc.vector.scalar_tensor_tensor(
                        prod[:, qcol], head_t[:, qcol], 1.0,
                        dec_bm[b][:, qcol],
                        op0=mult, op1=mult, accum_out=sq[q][:])
                nc.vector.tensor_add(sq[0][:], sq[0][:], sq[1][:])
                nc.vector.tensor_add(sq[2][:], sq[2][:], sq[3][:])
                nc.vector.tensor_add(s_col[:], sq[0][:], sq[2][:])
                nc.scalar.activation(e_sc, s_col[:], Exp,
                                     bias=negC[:], scale=1.0)

            # ---- finalize batch tile b right after its head-7 part ----
            if h == HEADS - 1:
                s_sum = scratch.tile([P, 1], f32, tag="ssum", name="ssum")
                rinv = scratch.tile([P, 1], f32, tag="rinv", name="rinv")
                nc.vector.reduce_sum(s_sum[:], e_all[b][:], axis=X)
                nc.vector.reciprocal(rinv[:], s_sum[:])
                out_f = fin.tile([P, HID], bf16, tag="outf", name="outf")
                brow = slice(b * P, (b + 1) * P)
                if not last:
                    # split the divide across ScalarE (has slack) and DVE
                    h0col = slice(0, NCHUNK)
                    h1col = slice(NCHUNK, HID)
                    nc.scalar.activation(out_f[:, h0col], oacc[b][:, h0col],
                                         Copy, scale=rinv[:])
                    nc.sync.dma_start(out_d[brow, h0col], out_f[:, h0col])
                    nc.vector.tensor_scalar(out_f[:, h1col],
                                            oacc[b][:, h1col],
                                            rinv[:], None, op0=mult)
                    nc.sync.dma_start(out_d[brow, h1col], out_f[:, h1col])
                else:
                    # fused: out = oacc*rinv + head7*(e7*rinv); divide on
                    # ScalarE, fused multiply-add on DVE, quartered so the
                    # two engines and the out-DMA pipeline
                    q7 = scratch.tile([P, 1], f32, tag="q7", name="q7")
                    nc.vector.tensor_scalar(q7[:], e_sc, rinv[:], None,
                                            op0=mult)
                    for q in range(4):
                        qcol = slice(q * QUARTER, (q + 1) * QUARTER)
                        nc.scalar.activation(out_f[:, qcol], oacc[b][:, qcol],
                                             Copy, scale=rinv[:])
                        nc.vector.scalar_tensor_tensor(
                            out_f[:, qcol], head_t[:, qcol], q7[:],
                            out_f[:, qcol], op0=mult, op1=add)
                        nc.sync.dma_start(out_d[brow, qcol], out_f[:, qcol])

    nc.compile()
    return nc


def _get_nc():
    if "nc" not in _cache:
        _cache["nc"] = _build()
    return _cache["nc"]


def build_in_maps(encoder_input, decoder_input, W_enc, b_enc, W_heads,
                  b_heads, W_dec, b_dec):
    import ml_dtypes
    bf = ml_dtypes.bfloat16

    def cast(a):
        return np.ascontiguousarray(np.asarray(a, dtype=np.float32)).astype(bf)

    xe = np.asarray(encoder_input, np.float32)     # [4096, 1024]
    xd = np.asarray(decoder_input, np.float32)     # [4096, 512]
    W_enc = np.asarray(W_enc, np.float32)
    W_dec = np.asarray(W_dec, np.float32)
    W_heads = np.asarray(W_heads, np.float32)

    we_p = cast(np.ascontiguousarray(
        W_enc.reshape(KT_E, P, HID).transpose(1, 0, 2)))           # [128,8,1024]
    wd_p = cast(np.ascontiguousarray(
        W_dec.reshape(KT_D, P, HID).transpose(1, 0, 2)))           # [128,4,1024]
    wh_p = cast(np.ascontiguousarray(
        W_heads.reshape(HEADS, KT_H, P, HID).transpose(0, 2, 1, 3)))  # [8,128,8,1024]

    benc_pp = np.ascontiguousarray(
        np.asarray(b_enc, np.float32).reshape(MT, P).T)            # [128, 8]
    bh_bc = cast(np.broadcast_to(
        np.asarray(b_heads, np.float32)[:, None, :], (HEADS, P, HID)))
    bd_bc = cast(np.broadcast_to(
        np.asarray(b_dec, np.float32)[None, :], (P, HID)))

    shared = {
        "we_p": we_p,
        "wd_p": wd_p,
        "wh_p": wh_p,
        "benc_pp": benc_pp,
        "bh_bc": bh_bc,
        "bd_bc": bd_bc,
    }
    in_maps = []
    for c in range(N_CORES):
        sl = slice(c * B_LOC, (c + 1) * B_LOC)
        m = dict(shared)
        m["xe_p"] = cast(np.ascontiguousarray(
            xe[sl].T.reshape(KT_E, P, B_LOC).transpose(1, 0, 2)))
        m["xd_p"] = cast(np.ascontiguousarray(
            xd[sl].T.reshape(KT_D, P, B_LOC).transpose(1, 0, 2)))
        in_maps.append(m)
    return in_maps


def kernel(encoder_input, decoder_input, W_enc, b_enc, W_heads, b_heads,
           W_dec, b_dec):
    from concourse.bass_utils import run_bass_kernel_spmd

    nc = _get_nc()
    in_maps = build_in_maps(encoder_input, decoder_input, W_enc, b_enc,
                            W_heads, b_heads, W_dec, b_dec)
    res = run_bass_kernel_spmd(nc, in_maps, list(range(N_CORES)))
    out = np.concatenate(
        [np.asarray(res.results[c]["out"]).astype(np.float32)
         for c in range(N_CORES)], axis=0)
    return out


# revision 39
# speedup vs baseline: 1.2026x; 1.0127x over previous
"""Trainium2 Bass kernel for nn_Attention2 (8-head encoder/decoder attention mix).

Reference computation (per full batch B=4096):
    enc_h  = relu(encoder_input @ W_enc + b_enc)               [B, 1024]
    heads  = relu(einsum('bh,khd->kbd', enc_h, W_heads) + b_heads)  [8, B, 1024]
    dec_H  = relu(decoder_input @ W_dec + b_dec)               [B, 1024]
    scores = sum(heads * dec_H, axis=2)                        [8, B]
    attn   = softmax(scores.T, axis=1)                         [B, 8]
    out    = einsum('kbd,bk->bd', heads, attn)                 [B, 1024]

Sharding: pure data-parallel over the batch dim across 8 NeuronCores
(B_loc = 512 per core, all params replicated, zero collectives).

Design notes (PE roofline: 608 matmuls of [128k x 128m x 512n] ~= 133us):
  - No bias matmuls: stage B/C PSUM tiles are PRE-FILLED with a broadcast
    bias row (ScalarE Copy for B, DVE copy for C) and the K-strip matmuls
    accumulate on top (start=False), pipelined 2 groups ahead of the
    drains so the PE never waits on ScalarE.
  - h-OUTER loop: W_heads (16 MB) streams during compute; head h is
    needed ~14.6us apart, far behind the ~355 GB/s aggregate DMA rate.
    All 8 heads resident in SBUF (128 KB/partition), fine-grained
    per-head tiles so head 0's matmuls only wait for head 0's DMA.
  - All bulk DMA rides the otherwise-idle sync queue in strict deadline
    order (queues share the 16 DMA engines with opaque arbitration, and
    a hw queue keeps only ~2 transfers in flight, so one ordered stream
    is both deterministic and near-optimal).
  - 8 junk warm-up matmuls during the initial DMA wait: they ramp the PE
    clock out of its low p-state AND touch every PSUM bank with a
    start=True matmul — banks used only by prefill+accumulate groups
    otherwise keep cold pending-zero state that silently drops the bias
    prefill on a cold core (caused an intermittent b-tile-0 error).
  - bf16 storage everywhere; measured-fast DVE ops (tensor_scalar 4x,
    tensor_tensor 2x; scalar_tensor_tensor 1x but fuses the score
    product+accumulate; tensor_tensor_reduce crashes the device - avoid).
  - Softmax finalize per b-tile interleaved into head 7 (divide split
    across ScalarE activation-scale and DVE) so only the last tile's
    ~4us chain trails the final matmul. Device output bf16 (host
    converts to f32), halving the output DMA.

Measured: 160.5us HW exec (baseline 176us), rel err 5.6e-3 (gate 2e-2).
"""

import numpy as np
from contextlib import ExitStack

N_CORES = 8
ENC_DIM, DEC_DIM, HID, HEADS, BATCH = 1024, 512, 1024, 8, 4096
B_LOC = BATCH // N_CORES          # 512 batch rows per core
P = 128                           # SBUF partitions
NCHUNK = 512                      # matmul moving free-dim (1 PSUM bank f32)
SCORE_SHIFT = 24.0                # scores measured in [14.2, 34.0]

KT_E = ENC_DIM // P               # 8 contraction tiles (enc dim)
KT_H = HID // P                   # 8 contraction tiles (hid dim)
KT_D = DEC_DIM // P               # 4 contraction tiles (dec dim)
MT = HID // P                     # 8 hid tiles (feature-major partitions)
BT = B_LOC // P                   # 4 batch tiles
NC_H = HID // NCHUNK              # 2 moving chunks over hid
QUARTER = 256                     # last-tile chain granularity

_cache = {}


def _build():
    import concourse.tile as tile
    from concourse import bacc, mybir

    f32 = mybir.dt.float32
    bf16 = mybir.dt.bfloat16
    Relu = mybir.ActivationFunctionType.Relu
    Exp = mybir.ActivationFunctionType.Exp
    Copy = mybir.ActivationFunctionType.Copy
    X = mybir.AxisListType.X
    mult = mybir.AluOpType.mult
    add = mybir.AluOpType.add
    vmax = mybir.AluOpType.max

    nc = bacc.Bacc("TRN2", target_bir_lowering=False, debug=False,
                   num_devices=N_CORES)

    xe_p = nc.dram_tensor("xe_p", [P, KT_E, B_LOC], bf16, kind="ExternalInput").ap()
    we_p = nc.dram_tensor("we_p", [P, KT_E, HID], bf16, kind="ExternalInput").ap()
    xd_p = nc.dram_tensor("xd_p", [P, KT_D, B_LOC], bf16, kind="ExternalInput").ap()
    wd_p = nc.dram_tensor("wd_p", [P, KT_D, HID], bf16, kind="ExternalInput").ap()
    wh_p = nc.dram_tensor("wh_p", [HEADS, P, KT_H, HID], bf16, kind="ExternalInput").ap()
    benc_pp = nc.dram_tensor("benc_pp", [P, MT], f32, kind="ExternalInput").ap()
    bh_bc = nc.dram_tensor("bh_bc", [HEADS, P, HID], bf16, kind="ExternalInput").ap()
    bd_bc = nc.dram_tensor("bd_bc", [P, HID], bf16, kind="ExternalInput").ap()
    out_d = nc.dram_tensor("out", [B_LOC, HID], bf16, kind="ExternalOutput").ap()

    with tile.TileContext(nc) as tc, ExitStack() as ctx:
        persist = ctx.enter_context(tc.tile_pool(name="persist", bufs=1))
        psums = ctx.enter_context(tc.tile_pool(name="psums", bufs=8, space="PSUM"))

        # --- persistent SBUF tiles ---
        WH = [persist.tile([P, KT_H, HID], bf16, tag=f"WH{h}", name=f"WH{h}")
              for h in range(HEADS)]
        BENC = persist.tile([P, MT], f32, tag="BENC", name="BENC")
        BD = persist.tile([P, HID], bf16, tag="BD", name="BD")
        BH = [persist.tile([P, HID], bf16, tag=f"BH{h}", name=f"BH{h}")
              for h in range(HEADS)]
        negC = persist.tile([P, 1], f32, tag="negC", name="negC")
        junk = persist.tile([P, NCHUNK], bf16, tag="junk", name="junk")
        nc.vector.memset(negC[:], -SCORE_SHIFT)
        nc.vector.memset(junk[:], 0.5)
        ench = [persist.tile([P, B_LOC], bf16, tag=f"ench{m}", name=f"ench{m}")
                for m in range(MT)]
        dec_bm = [persist.tile([P, HID], bf16, tag=f"dec{b}", name=f"dec{b}")
                  for b in range(BT)]
        e_all = [persist.tile([P, HEADS], f32, tag=f"eall{b}", name=f"eall{b}")
                 for b in range(BT)]
        oacc = [persist.tile([P, HID], bf16, tag=f"oacc{b}", name=f"oacc{b}")
                for b in range(BT)]

        poolA = tc.tile_pool(name="poolA", bufs=1)
        pa = poolA.__enter__()
        XEh = [pa.tile([P, 2, B_LOC], bf16, tag=f"XEh{i}", name=f"XEh{i}")
               for i in range(4)]
        WEh = [pa.tile([P, 2, HID], bf16, tag=f"WEh{i}", name=f"WEh{i}")
               for i in range(4)]
        XD = pa.tile([P, KT_D, B_LOC], bf16, tag="XD", name="XD")
        WD = pa.tile([P, KT_D, HID], bf16, tag="WD", name="WD")

        def XEk(k):
            return XEh[k // 2][:, k % 2, :]

        def WEk(k):
            return WEh[k // 2][:, k % 2, :]

        # --- t=0 DMA issue: one queue (sync) in strict global deadline
        # order — queues race for the same ~355 GB/s of shared DMA
        # engines with opaque arbitration, so one ordered stream is both
        # deterministic and optimal. Only the tiny BENC rides scalar.
        nc.scalar.dma_start(BENC[:], benc_pp)
        for i in range(4):
            nc.sync.dma_start(XEh[i][:], xe_p[:, 2 * i:2 * i + 2, :])
            nc.sync.dma_start(WEh[i][:], we_p[:, 2 * i:2 * i + 2, :])
        nc.sync.dma_start(XD[:], xd_p)
        nc.sync.dma_start(WD[:], wd_p)
        nc.sync.dma_start(BD[:], bd_bc)
        nc.sync.dma_start(BH[0][:], bh_bc[0])
        nc.sync.dma_start(BH[1][:], bh_bc[1])
        nc.sync.dma_start(WH[0][:], wh_p[0])
        nc.sync.dma_start(BH[2][:], bh_bc[2])
        nc.sync.dma_start(BH[3][:], bh_bc[3])
        nc.sync.dma_start(WH[1][:], wh_p[1])
        nc.sync.dma_start(WH[2][:], wh_p[2])
        for h in range(4, HEADS):
            nc.sync.dma_start(BH[h][:], bh_bc[h])
        for h in range(3, HEADS):
            nc.sync.dma_start(WH[h][:], wh_p[h])

        # --- PE p-state warm-up on junk data during the DMA wait. One
        # start=True matmul per PSUM bank: banks later used ONLY by
        # prefill+accumulate groups otherwise keep cold pending-zero
        # state that silently drops the bias prefill on first use.
        for i in range(8):
            wps = psums.tile([P, NCHUNK], f32, tag="mm", name="ps")
            nc.tensor.matmul(wps[:], junk[:, :P], junk[:],
                             start=True, stop=True)

        # ---- group list: stage C (8 groups), then stage B h-outer ----
        groups = []
        for b in range(BT):
            for n in range(NC_H):
                groups.append(("C", None, b, n))
        for h in range(HEADS):
            for b in range(BT):
                for n in range(NC_H):
                    groups.append(("B", h, b, n))
        NG = len(groups)
        ps_of = {}

        def prefill(g):
            kind, h, b, n = groups[g]
            ps = psums.tile([P, NCHUNK], f32, tag="mm", name="ps")
            ncol = slice(n * NCHUNK, (n + 1) * NCHUNK)
            if kind == "C":
                nc.vector.tensor_copy(ps[:], BD[:, ncol])
            else:
                nc.scalar.activation(ps[:], BH[h][:, ncol], Copy)
            ps_of[g] = ps

        def emit_matmuls(g):
            kind, h, b, n = groups[g]
            ps = ps_of[g]
            ncol = slice(n * NCHUNK, (n + 1) * NCHUNK)
            bcol = slice(b * P, (b + 1) * P)
            if kind == "C":
                for k in range(KT_D):
                    nc.tensor.matmul(ps[:], XD[:, k, bcol], WD[:, k, ncol],
                                     start=False, stop=(k == KT_D - 1),
                                     skip_group_check=True)
            else:
                for k in range(KT_H):
                    nc.tensor.matmul(ps[:], ench[k][:, bcol],
                                     WH[h][:, k, ncol],
                                     start=False, stop=(k == KT_H - 1),
                                     skip_group_check=True)

        # ---- Stage A: enc trunk, feature-major, 2 waves of 4 m-tiles ----
        for wave in range(2):
            mset = range(wave * MT // 2, (wave + 1) * MT // 2)
            pss = {}
            for m in mset:
                pss[m] = psums.tile([P, B_LOC], f32, tag="mm", name="ps")
            for k in range(KT_E):
                for m in mset:
                    nc.tensor.matmul(pss[m][:], WEk(k)[:, m * P:(m + 1) * P],
                                     XEk(k),
                                     start=(k == 0), stop=(k == KT_E - 1))
            for m in mset:
                nc.scalar.activation(ench[m][:], pss[m][:], Relu,
                                     bias=BENC[:, m:m + 1], scale=1.0)
            if wave == 0:
                prefill(0)
                prefill(1)

        # ---- Stage C: DVE handles bias prefill + relu drain ----
        for g in range(BT * NC_H):
            kind, h, b, n = groups[g]
            ncol = slice(n * NCHUNK, (n + 1) * NCHUNK)
            emit_matmuls(g)
            if g + 2 < NG:
                prefill(g + 2)
            ps_cur = ps_of.pop(g)
            nc.vector.tensor_scalar(dec_bm[b][:, ncol], ps_cur[:], 0.0, None,
                                    op0=vmax)

        poolA.__exit__(None, None, None)

        # ---- Stage B: h-outer; finalize interleaved into head 7 ----
        head_pool = ctx.enter_context(tc.tile_pool(name="head", bufs=3))
        scratch = ctx.enter_context(tc.tile_pool(name="scratch", bufs=4))
        fin = ctx.enter_context(tc.tile_pool(name="fin", bufs=2))

        head_t = None
        for g in range(BT * NC_H, NG):
            kind, h, b, n = groups[g]
            ncol = slice(n * NCHUNK, (n + 1) * NCHUNK)
            last = h == HEADS - 1 and b == BT - 1
            emit_matmuls(g)
            if g + 2 < NG:
                prefill(g + 2)
            ps_cur = ps_of.pop(g)
            if n == 0:
                head_t = head_pool.tile([P, HID], bf16, tag="head", name="head")
            if not (last and n == NC_H - 1):
                nc.scalar.activation(head_t[:, ncol], ps_cur[:], Relu)
            else:
                # very last chunk: relu in quarters so the score chain
                # starts earlier
                for q in range(2):
                    qcol = slice(NCHUNK + q * QUARTER, NCHUNK + (q + 1) * QUARTER)
                    pcol = slice(q * QUARTER, (q + 1) * QUARTER)
                    nc.scalar.activation(head_t[:, qcol], ps_cur[:, pcol], Relu)
            if n != NC_H - 1:
                continue

            # ---- score + exp + out-accumulate for (h, b) ----
            prod = scratch.tile([P, HID], bf16, tag="prod", name="prod")
            s_col = scratch.tile([P, 1], f32, tag="scol", name="scol")
            e_sc = e_all[b][:, h:h + 1]
            if not last:
                nc.vector.scalar_tensor_tensor(
                    prod[:], head_t[:], 1.0, dec_bm[b][:],
                    op0=mult, op1=mult, accum_out=s_col[:])
                nc.scalar.activation(e_sc, s_col[:], Exp,
                                     bias=negC[:], scale=1.0)
                if h == 0:
                    nc.vector.tensor_scalar(oacc[b][:], head_t[:], e_sc, None,
                                            op0=mult)
                else:
                    nc.vector.scalar_tensor_tensor(
                        oacc[b][:], head_t[:], e_sc, oacc[b][:],
                        op0=mult, op1=add)
            else:
                # last (h,b): quartered score, then fused finalize
                sq = [scratch.tile([P, 1], f32, tag=f"sq{q}", name=f"sq{q}")
                      for q in range(4)]
                for q in range(4):
                    qcol = slice(q * QUARTER, (q + 1) * QUARTER)
                    nc.vector.scalar_tensor_tensor(
                        prod[:, qcol], head_t[:, qcol], 1.0,
                        dec_bm[b][:, qcol],
                        op0=mult, op1=mult, accum_out=sq[q][:])
                nc.vector.tensor_add(sq[0][:], sq[0][:], sq[1][:])
                nc.vector.tensor_add(sq[2][:], sq[2][:], sq[3][:])
                nc.vector.tensor_add(s_col[:], sq[0][:], sq[2][:])
                nc.scalar.activation(e_sc, s_col[:], Exp,
                                     bias=negC[:], scale=1.0)

            # ---- finalize batch tile b right after its head-7 part ----
            if h == HEADS - 1:
                s_sum = scratch.tile([P, 1], f32, tag="ssum", name="ssum")
                rinv = scratch.tile([P, 1], f32, tag="rinv", name="rinv")
                nc.vector.reduce_sum(s_sum[:], e_all[b][:], axis=X)
                nc.vector.reciprocal(rinv[:], s_sum[:])
                out_f = fin.tile([P, HID], bf16, tag="outf", name="outf")
                brow = slice(b * P, (b + 1) * P)
                if not last:
                    # split the divide across ScalarE (has slack) and DVE
                    h0col = slice(0, NCHUNK)
                    h1col = slice(NCHUNK, HID)
                    nc.scalar.activation(out_f[:, h0col], oacc[b][:, h0col],
                                         Copy, scale=rinv[:])
                    nc.sync.dma_start(out_d[brow, h0col], out_f[:, h0col])
                    nc.vector.tensor_scalar(out_f[:, h1col],
                                            oacc[b][:, h1col],
                                            rinv[:], None, op0=mult)
                    nc.sync.dma_start(out_d[brow, h1col], out_f[:, h1col])
                else:
                    # fused: out = oacc*rinv + head7*(e7*rinv); divide on
                    # ScalarE, fused multiply-add on DVE, quartered so the
                    # two engines and the out-DMA pipeline
                    q7 = scratch.tile([P, 1], f32, tag="q7", name="q7")
                    nc.vector.tensor_scalar(q7[:], e_sc, rinv[:], None,
                                            op0=mult)
                    for q in range(4):
                        qcol = slice(q * QUARTER, (q + 1) * QUARTER)
                        nc.scalar.activation(out_f[:, qcol], oacc[b][:, qcol],
                                             Copy, scale=rinv[:])
                        nc.vector.scalar_tensor_tensor(
                            out_f[:, qcol], head_t[:, qcol], q7[:],
                            out_f[:, qcol], op0=mult, op1=add)
                        nc.sync.dma_start(out_d[brow, qcol], out_f[:, qcol])

    nc.compile()
    return nc


def _get_nc():
    if "nc" not in _cache:
        _cache["nc"] = _build()
    return _cache["nc"]


def build_in_maps(encoder_input, decoder_input, W_enc, b_enc, W_heads,
                  b_heads, W_dec, b_dec):
    import ml_dtypes
    bf = ml_dtypes.bfloat16

    def cast(a):
        return np.ascontiguousarray(np.asarray(a, dtype=np.float32)).astype(bf)

    xe = np.asarray(encoder_input, np.float32)     # [4096, 1024]
    xd = np.asarray(decoder_input, np.float32)     # [4096, 512]
    W_enc = np.asarray(W_enc, np.float32)
    W_dec = np.asarray(W_dec, np.float32)
    W_heads = np.asarray(W_heads, np.float32)

    we_p = cast(np.ascontiguousarray(
        W_enc.reshape(KT_E, P, HID).transpose(1, 0, 2)))           # [128,8,1024]
    wd_p = cast(np.ascontiguousarray(
        W_dec.reshape(KT_D, P, HID).transpose(1, 0, 2)))           # [128,4,1024]
    wh_p = cast(np.ascontiguousarray(
        W_heads.reshape(HEADS, KT_H, P, HID).transpose(0, 2, 1, 3)))  # [8,128,8,1024]

    benc_pp = np.ascontiguousarray(
        np.asarray(b_enc, np.float32).reshape(MT, P).T)            # [128, 8]
    bh_bc = cast(np.broadcast_to(
        np.asarray(b_heads, np.float32)[:, None, :], (HEADS, P, HID)))
    bd_bc = cast(np.broadcast_to(
        np.asarray(b_dec, np.float32)[None, :], (P, HID)))

    shared = {
        "we_p": we_p,
        "wd_p": wd_p,
        "wh_p": wh_p,
        "benc_pp": benc_pp,
        "bh_bc": bh_bc,
        "bd_bc": bd_bc,
    }
    in_maps = []
    for c in range(N_CORES):
        sl = slice(c * B_LOC, (c + 1) * B_LOC)
        m = dict(shared)
        m["xe_p"] = cast(np.ascontiguousarray(
            xe[sl].T.reshape(KT_E, P, B_LOC).transpose(1, 0, 2)))
        m["xd_p"] = cast(np.ascontiguousarray(
            xd[sl].T.reshape(KT_D, P, B_LOC).transpose(1, 0, 2)))
        in_maps.append(m)
    return in_maps


def kernel(encoder_input, decoder_input, W_enc, b_enc, W_heads, b_heads,
           W_dec, b_dec):
    from concourse.bass_utils import run_bass_kernel_spmd

    nc = _get_nc()
    in_maps = build_in_maps(encoder_input, decoder_input, W_enc, b_enc,
                            W_heads, b_heads, W_dec, b_dec)
    res = run_bass_kernel_spmd(nc, in_maps, list(range(N_CORES)))
    out = np.concatenate(
        [np.asarray(res.results[c]["out"]).astype(np.float32)
         for c in range(N_CORES)], axis=0)
    return out
